# revision 14
# baseline (speedup 1.0000x reference)
"""MoE gate kernel for Trainium2 (8 NeuronCores).

reference math: logits = x @ W_g; probs = softmax(logits); top-8 (vals, ids).

Strategy (token-parallel, 2048 tokens/core, default variant "pt32c"):
  - x is reformatted host-side into a transposed layout
    XT[g, p, c, t] = x[g*TG + t, c*128 + p]  so the device streams x^T
    tiles [128 d, NDC, TG=256] with plain contiguous DMA (1 MiB sub-DMAs
    on alternating sync/scalar HWDGE rings, 4-deep group prefetch, no
    on-chip transposes at all).
  - fp32 PE gemm (exact), accumulated over 32 k-chunks in PSUM, using
    128x64 column tiling: the PE array is split into two independent
    64-col tiles (tile_position (0,0)/(0,64)), each holding a 64-token
    stationary x-chunk; both stream the same moving W [128,64]
    concurrently, doubling throughput of the fp32 HI/LO pair stream.
    Per token the contraction order is identical to the full-array
    gemm="x" -> bit-exact vs the jax reference (0 id mismatches).
  - top-8 selection on exact fp32 logits via DVE max8/max_index
  - vals = exp(top8_logit - max) * 1/sum(exp(logits - max))
"""
import os
import sys
sys.path.insert(0, "/opt/trn_rl_repo")
import numpy as np

N_TOKENS = 16384
D = 4096
E = 64
TOPK = 8
N_CORES = 8
T_CORE = N_TOKENS // N_CORES   # 2048
TG = int(os.environ.get("MOE_TG", "256"))  # tokens per group
N_GROUPS = T_CORE // TG        # 4
TPG = TG // 128                # token-tiles per group
NDC = D // 128                 # 32 k-chunks

_cache = {}


SXL = 4          # log2 scale for x in the f16 split
SWL = 10         # log2 scale for W_g in the f16 split
RESCALE = 2.0 ** (-(SXL + SWL))


def build_tx(reps: int = 1, internal_x: bool = False, mode: str = "full",
             gemm: str = "x", prec: str = "f32", layout: str = "plain"):
    import concourse.mybir as mybir
    import concourse.tile as tile
    from concourse import bacc
    from concourse.bass import ds
    from concourse.masks import make_identity

    dt = mybir.dt
    F32 = dt.float32
    F16 = dt.float16
    AF = mybir.ActivationFunctionType
    AX = mybir.AxisListType
    ALU = mybir.AluOpType

    XTS_BUFS = int(os.environ.get("XTS_BUFS", "4"))
    G_BUFS = int(os.environ.get("G_BUFS", "2"))
    LT_BUFS = int(os.environ.get("LT_BUFS", "2"))
    DMA_SPLIT = int(os.environ.get("DMA_SPLIT", "4"))  # DMAs per group
    # dummy 64-col matmuls appended per group: keep the PE HAM-warm across
    # DMA-paced group boundaries instead of micro-idling (throttle to 1.2GHz)
    FILL_MM = int(os.environ.get("FILL_MM", "0"))

    nc = bacc.Bacc("TRN2", target_bir_lowering=False, debug=False)
    SPL = DMA_SPLIT
    CW = NDC // SPL
    xshape = [N_GROUPS, SPL, 128, CW, TG]
    if layout == "xbar":
        U16 = dt.uint16
        if internal_x:
            xu_d = nc.dram_tensor("xuint", [NDC, 2 * T_CORE, 128], U16)
        else:
            xu_d = nc.dram_tensor("xu", [NDC, 2 * T_CORE, 128], U16,
                                  kind="ExternalInput")
        w_d = nc.dram_tensor("w", [D, E], F32, kind="ExternalInput")
    elif prec == "f16":
        kind = {} if internal_x else {"kind": "ExternalInput"}
        sfx = "int" if internal_x else ""
        xh_d = nc.dram_tensor("xh" + sfx, xshape, F16, **kind)
        xl_d = nc.dram_tensor("xl" + sfx, xshape, F16, **kind)
        wh_d = nc.dram_tensor("wh", [D, E], F16, kind="ExternalInput")
        wl_d = nc.dram_tensor("wl", [D, E], F16, kind="ExternalInput")
    else:
        if internal_x:
            xt_d = nc.dram_tensor("xtint", xshape, F32)
        else:
            xt_d = nc.dram_tensor("xt", xshape, F32, kind="ExternalInput")
        w_d = nc.dram_tensor("w", [D, E], F32, kind="ExternalInput")
    ids_d = nc.dram_tensor("ids", [T_CORE, TOPK], dt.uint32, kind="ExternalOutput")
    vals_d = nc.dram_tensor("vals", [T_CORE, TOPK], F32, kind="ExternalOutput")

    with tile.TileContext(nc) as tc:
        if mode == "compute":
            tc.race_detector_enabled = False
        with (
            tc.tile_pool(name="xts", bufs=XTS_BUFS) as xts_pool,
            tc.tile_pool(name="wp", bufs=1) as w_pool,
            tc.tile_pool(name="lf", bufs=2) as lf_pool,
            tc.tile_pool(name="sm", bufs=2) as sm_pool,
            tc.tile_pool(name="outp", bufs=1) as out_pool,
            tc.tile_pool(name="gp", bufs=G_BUFS, space="PSUM") as g_psum,
            tc.tile_pool(name="lt", bufs=LT_BUFS, space="PSUM") as lt_psum,
        ):
            ident = w_pool.tile([128, 128], F32, tag="ident")
            make_identity(nc, ident)
            if gemm == "c" and FILL_MM:
                fz = w_pool.tile([128, 64], F32, tag="fz")
                nc.vector.memset(fz[:], 0.0)
            if prec == "f16":
                wh_sb = w_pool.tile([128, NDC, E], F16, tag="wh")
                nc.gpsimd.dma_start(
                    wh_sb[:], wh_d.rearrange("(c p) e -> p c e", p=128))
                wl_sb = w_pool.tile([128, NDC, E], F16, tag="wl")
                nc.gpsimd.dma_start(
                    wl_sb[:], wl_d.rearrange("(c p) e -> p c e", p=128))
            else:
                w_sb = w_pool.tile([128, NDC, E], F32, tag="w")
                nc.gpsimd.dma_start(
                    w_sb[:], w_d.rearrange("(c p) e -> p c e", p=128))

            i_all = out_pool.tile([128, T_CORE // 128, TOPK], dt.uint32, tag="i")
            v_all = out_pool.tile([128, T_CORE // 128, TOPK], F32, tag="v")

            def softmax_top8(pl, idx):
                """pl: PSUM [128 tok, 64 exp] exact fp32 logits."""
                l_sb = sm_pool.tile([128, E], F32, tag="l")
                nc.vector.tensor_copy(l_sb[:], pl[:])
                nmax = sm_pool.tile([128, 1], F32, tag="nm")
                nc.vector.tensor_reduce(
                    nmax[:], l_sb[:], axis=AX.X, op=ALU.max, negate=True,
                )
                e_sb = sm_pool.tile([128, E], F32, tag="e")
                s_sb = sm_pool.tile([128, 1], F32, tag="s")
                nc.scalar.activation(
                    e_sb[:], pl[:], AF.Exp, bias=nmax[:], accum_out=s_sb[:],
                )
                r_sb = sm_pool.tile([128, 1], F32, tag="r")
                nc.vector.reciprocal(r_sb[:], s_sb[:])
                m8 = sm_pool.tile([128, TOPK], F32, tag="m8")
                nc.vector.max(out=m8[:], in_=l_sb[:])
                nc.vector.max_index(
                    out=i_all[:, idx, :], in_max=m8[:], in_values=l_sb[:],
                )
                e8 = sm_pool.tile([128, TOPK], F32, tag="e8")
                nc.scalar.activation(e8[:], m8[:], AF.Exp, bias=nmax[:])
                nc.vector.tensor_scalar(
                    out=v_all[:, idx, :], in0=e8[:], scalar1=r_sb[:],
                    scalar2=None, op0=ALU.mult,
                )

            RINGS = os.environ.get("RINGS", "sc")
            _ENGS = {"s": nc.sync, "c": nc.scalar, "3": nc.gpsimd}

            def load_group(g, tile_, dram, parity):
                for s in range(SPL):
                    eng = _ENGS[RINGS[(g * SPL + s + parity) % len(RINGS)]]
                    eng.dma_start(
                        tile_[:, ds(s * CW, CW), :],
                        dram[g, s, :, :, :],
                    )

            def issue_load(g):
                """Allocate + start this group's loads.  Called one group
                AHEAD of compute so the DMA instructions land on the sync/
                scalar queues BEFORE the previous group's tail (ACT exp)
                instructions — otherwise the scalar-ring DMAs sit FIFO
                behind gemm-dependent activations and DMA serializes with
                compute."""
                if prec == "f16":
                    xtsh = xts_pool.tile([128, NDC, TG], F16, tag="xh")
                    xtsl = xts_pool.tile([128, NDC, TG], F16, tag="xl")
                    if mode != "compute":
                        load_group(g, xtsh, xh_d, g)
                        load_group(g, xtsl, xl_d, g + 1)
                    else:
                        nc.vector.memset(xtsh[:, 0, ds(0, 4)], 0.0)
                        nc.vector.memset(xtsl[:, 0, ds(0, 4)], 0.0)
                    return (xtsh, xtsl)
                xts = xts_pool.tile([128, NDC, TG], F32, tag="xts")
                if mode == "compute":
                    nc.vector.memset(xts[:, 0, ds(0, 4)], 0.0)
                elif layout == "xbar":
                    for c in range(NDC):
                        # all xbar transposes on ONE HWDGE ring: concurrent
                        # transposes from sync+scalar rings corrupt data
                        nc.sync.dma_start(
                            xts[:, c, :].bitcast(dt.uint16),
                            xu_d[c, ds(2 * g * TG, 2 * TG), :],
                            transpose=True,
                        )
                else:
                    load_group(g, xts, xt_d, g)
                return xts

            def body():
                pend = issue_load(0)
                for g in range(N_GROUPS):
                    cur = pend
                    if g + 1 < N_GROUPS:
                        pend = issue_load(g + 1)
                    if prec == "f16":
                        xtsh, xtsl = cur
                        if mode == "dma":
                            continue
                        pg = g_psum.tile([64, TG], F32, tag="g")
                        n_mm = NDC * 3
                        i_mm = 0
                        for dc in range(NDC):
                            for (wt, xt_t) in ((wh_sb, xtsh), (wh_sb, xtsl),
                                               (wl_sb, xtsh)):
                                nc.tensor.matmul(
                                    pg[:], wt[:, dc, :], xt_t[:, dc, :],
                                    start=(i_mm == 0), stop=(i_mm == n_mm - 1),
                                )
                                i_mm += 1
                        lf_sb = lf_pool.tile([64, TG], F32, tag="lf")
                        nc.vector.tensor_scalar(
                            out=lf_sb[:], in0=pg[:], scalar1=RESCALE,
                            scalar2=None, op0=ALU.mult,
                        )
                        for tt in range(TPG):
                            pl = lt_psum.tile([128, E], F32, tag="lt")
                            nc.tensor.matmul(
                                pl[:], lf_sb[:, ds(tt * 128, 128)],
                                ident[:64, :64], is_transpose=True,
                            )
                            softmax_top8(pl, g * TPG + tt)
                        continue
                    xts = cur
                    if mode == "dma":
                        continue
                    if gemm == "x":
                        for tt in range(TPG):
                            pa = g_psum.tile([128, E], F32, tag=f"pa{tt % 2}")
                            for dc in range(NDC):
                                nc.tensor.matmul(
                                    pa[:], xts[:, dc, ds(tt * 128, 128)],
                                    w_sb[:, dc, :],
                                    start=(dc == 0), stop=(dc == NDC - 1),
                                )
                            softmax_top8(pa, g * TPG + tt)
                    elif gemm == "c":
                        # 128x64 column tiling: two independent PE tiles
                        # (cols 0-63 / 64-127) stream W concurrently, one
                        # 64-token stationary half each.  Same per-token
                        # contraction order as gemm="x" -> bit-exact.
                        for tt in range(TPG):
                            pa = g_psum.tile([128, E], F32, tag=f"pa{tt % 2}")
                            for dc in range(NDC):
                                for h in range(2):
                                    nc.tensor.matmul(
                                        pa[h * 64:(h + 1) * 64, :],
                                        xts[:, dc, ds(tt * 128 + h * 64, 64)],
                                        w_sb[:, dc, :],
                                        start=(dc == 0),
                                        stop=(dc == NDC - 1),
                                        tile_position=(0, h * 64),
                                    )
                            softmax_top8(pa, g * TPG + tt)
                        if FILL_MM:
                            fpa = lt_psum.tile([128, 64], F32, tag="fpa")
                            for _ in range(FILL_MM):
                                nc.tensor.matmul(
                                    fpa[:64, :], fz[:, :], fz[:, :],
                                    start=True, stop=True,
                                    tile_position=(0, 0),
                                )
                    else:
                        pg = g_psum.tile([64, TG], F32, tag="g")
                        for dc in range(NDC):
                            nc.tensor.matmul(
                                pg[:], w_sb[:, dc, :], xts[:, dc, :],
                                start=(dc == 0), stop=(dc == NDC - 1),
                            )
                        lf_sb = lf_pool.tile([64, TG], F32, tag="lf")
                        nc.vector.tensor_copy(lf_sb[:], pg[:])
                        for tt in range(TPG):
                            pl = lt_psum.tile([128, E], F32, tag="lt")
                            nc.tensor.matmul(
                                pl[:], lf_sb[:, ds(tt * 128, 128)],
                                ident[:64, :64], is_transpose=True,
                            )
                            softmax_top8(pl, g * TPG + tt)
                if mode == "dma":
                    nc.vector.memset(i_all[:], 0)
                    nc.vector.memset(v_all[:], 0.0)
                nc.sync.dma_start(
                    ids_d.rearrange("(q p) k -> p q k", p=128), i_all[:]
                )
                nc.sync.dma_start(
                    vals_d.rearrange("(q p) k -> p q k", p=128), v_all[:]
                )

            if reps == 1:
                body()
            else:
                with tc.For_i(0, reps, 1):
                    body()

    nc.finalize()
    return nc


# Default "pt32c": host-transposed plain-DMA layout + 128x64 PE column
# tiling (two tiles stream W concurrently, one 64-token stationary half
# each).  Bit-exact vs the reference (0 id mismatches, 0.0 rel err): each
# token's dot product keeps the exact gemm="x" contraction order.
# ~121 us/rep under heavy co-tenant load vs 250+ for the old tx32 default;
# ~100-120 us on a quiet device (DMA floor 32MiB @ ~360-380 GB/s ~ 89-94 us).
# Alternatives (env MOE_VARIANT):
#   tx32  — device DMA-transpose layout, bit-exact, but single-ring
#           transpose DMA caps at ~261 GB/s and PE stalls throttle HAM
#   pt32  — plain-DMA layout, bit-exact full-array gemm; PE-bound at
#           1024 fp32 HI/LO pair instrs/rep (~109 us warm, ~218 throttled)
#   pt32w — W-stationary fp32 gemm, NOT bit-identical to the reference
#           (flips the order of two half-ulp-tied experts on 1/16384
#           tokens; vals rel err ~2e-6)
#   pt16  — fp16 hi/lo 3-term gemm, same single tie-token caveat
VARIANT = os.environ.get("MOE_VARIANT", "pt32c")


def _get_nc(reps: int = 1, internal_x: bool = False, mode: str = "full",
            variant: str | None = None):
    variant = variant or VARIANT
    key = (reps, internal_x, mode, variant)
    if key not in _cache:
        gemm = "w" if variant.endswith("w") else ("c" if variant.endswith("c") else "x")
        prec = "f16" if variant == "pt16" else "f32"
        layout = "xbar" if variant.startswith("tx") else "plain"
        _cache[key] = build_tx(reps, internal_x, mode, gemm=gemm, prec=prec,
                               layout=layout)
    return _cache[key]


def _transpose_xt(x5: np.ndarray) -> np.ndarray:
    """[N_CORES, N_GROUPS, TG, NDC, 128] -> [N_CORES, N_GROUPS, SPL, 128, CW, TG]."""
    spl = int(os.environ.get("DMA_SPLIT", "4"))
    x6 = x5.reshape(N_CORES, N_GROUPS, TG, spl, NDC // spl, 128)
    return np.ascontiguousarray(x6.transpose(0, 1, 3, 5, 4, 2))


def bench_in_maps(w: np.ndarray):
    """in_maps for the internal-x timed variant (x DRAM tensors internal)."""
    w = np.ascontiguousarray(np.asarray(w), dtype=np.float32)
    if VARIANT == "pt16":
        ws = w * (2.0 ** SWL)
        wh = ws.astype(np.float16)
        wl = (ws - wh.astype(np.float32)).astype(np.float16)
        return [{"wh": wh, "wl": wl} for _ in range(N_CORES)]
    return [{"w": w} for _ in range(N_CORES)]


def _to_xu(x: np.ndarray) -> np.ndarray:
    """[N_TOKENS, D] f32 -> [N_CORES, NDC, 2*T_CORE, 128] u16 interleaved."""
    xv = x.view(np.uint16).reshape(N_CORES, T_CORE, NDC, 128, 2)
    return np.ascontiguousarray(
        xv.transpose(0, 2, 1, 4, 3).reshape(N_CORES, NDC, 2 * T_CORE, 128)
    )


def kernel(x: np.ndarray, W_g: np.ndarray):
    from concourse.bass_utils import run_bass_kernel_spmd

    x = np.ascontiguousarray(np.asarray(x), dtype=np.float32)
    w = np.ascontiguousarray(np.asarray(W_g), dtype=np.float32)
    nc = _get_nc(1)
    if VARIANT.startswith("tx"):
        xu = _to_xu(x)
        in_maps = [{"xu": xu[c], "w": w} for c in range(N_CORES)]
        res = run_bass_kernel_spmd(nc, in_maps, core_ids=list(range(N_CORES)))
        ids = np.concatenate([res.results[c]["ids"] for c in range(N_CORES)], axis=0)
        vals = np.concatenate([res.results[c]["vals"] for c in range(N_CORES)], axis=0)
        return ids.astype(np.int32), vals
    x5 = x.reshape(N_CORES, N_GROUPS, TG, NDC, 128)
    if VARIANT == "pt16":
        xs = x5 * (2.0 ** SXL)
        xh = xs.astype(np.float16)
        xl = (xs - xh.astype(np.float32)).astype(np.float16)
        xht = _transpose_xt(xh)
        xlt = _transpose_xt(xl)
        ws = w * (2.0 ** SWL)
        wh = ws.astype(np.float16)
        wl = (ws - wh.astype(np.float32)).astype(np.float16)
        in_maps = [
            {"xh": xht[c], "xl": xlt[c], "wh": wh, "wl": wl}
            for c in range(N_CORES)
        ]
    else:
        xt = _transpose_xt(x5)
        in_maps = [{"xt": xt[c], "w": w} for c in range(N_CORES)]
    res = run_bass_kernel_spmd(nc, in_maps, core_ids=list(range(N_CORES)))
    ids = np.concatenate([res.results[c]["ids"] for c in range(N_CORES)], axis=0)
    vals = np.concatenate([res.results[c]["vals"] for c in range(N_CORES)], axis=0)
    return ids.astype(np.int32), vals



# revision 19
# speedup vs baseline: 1.0140x; 1.0140x over previous
"""MoE gate kernel for Trainium2 (8 NeuronCores).

reference math: logits = x @ W_g; probs = softmax(logits); top-8 (vals, ids).

Strategy (token-parallel, 2048 tokens/core, default variant "pt32c"):
  - x is reformatted host-side into a transposed layout
    XT[g, p, c, t] = x[g*TG + t, c*128 + p]  so the device streams x^T
    tiles [128 d, NDC, TG=256] with plain contiguous DMA (1 MiB sub-DMAs
    on alternating sync/scalar HWDGE rings, 4-deep group prefetch, no
    on-chip transposes at all).
  - fp32 PE gemm (exact), accumulated over 32 k-chunks in PSUM, using
    128x64 column tiling: the PE array is split into two independent
    64-col tiles (tile_position (0,0)/(0,64)), each holding a 64-token
    stationary x-chunk; both stream the same moving W [128,64]
    concurrently, doubling throughput of the fp32 HI/LO pair stream.
    Per token the contraction order is identical to the full-array
    gemm="x" -> bit-exact vs the jax reference (0 id mismatches).
  - top-8 selection on exact fp32 logits via DVE max8/max_index
  - vals = exp(top8_logit - max) * 1/sum(exp(logits - max))
"""
import os
import sys
sys.path.insert(0, "/opt/trn_rl_repo")
import numpy as np

N_TOKENS = 16384
D = 4096
E = 64
TOPK = 8
N_CORES = 8
T_CORE = N_TOKENS // N_CORES   # 2048
TG = int(os.environ.get("MOE_TG", "256"))  # tokens per group
N_GROUPS = T_CORE // TG        # 4
TPG = TG // 128                # token-tiles per group
NDC = D // 128                 # 32 k-chunks

_cache = {}


SXL = 4          # log2 scale for x in the f16 split
SWL = 10         # log2 scale for W_g in the f16 split
RESCALE = 2.0 ** (-(SXL + SWL))


def build_tx(reps: int = 1, internal_x: bool = False, mode: str = "full",
             gemm: str = "x", prec: str = "f32", layout: str = "plain"):
    import concourse.mybir as mybir
    import concourse.tile as tile
    from concourse import bacc
    from concourse.bass import ds
    from concourse.masks import make_identity

    dt = mybir.dt
    F32 = dt.float32
    F16 = dt.float16
    AF = mybir.ActivationFunctionType
    AX = mybir.AxisListType
    ALU = mybir.AluOpType

    XTS_BUFS = int(os.environ.get("XTS_BUFS", "4"))
    G_BUFS = int(os.environ.get("G_BUFS", "2"))
    LT_BUFS = int(os.environ.get("LT_BUFS", "2"))
    DMA_SPLIT = int(os.environ.get("DMA_SPLIT", "4"))  # DMAs per group
    # dummy 64-col matmuls appended per group: keep the PE HAM-warm across
    # DMA-paced group boundaries instead of micro-idling (throttle to 1.2GHz)
    FILL_MM = int(os.environ.get("FILL_MM", "0"))

    nc = bacc.Bacc("TRN2", target_bir_lowering=False, debug=False)
    SPL = DMA_SPLIT
    CW = NDC // SPL
    xshape = [N_GROUPS, SPL, 128, CW, TG]
    if layout == "xbar":
        U16 = dt.uint16
        if internal_x:
            xu_d = nc.dram_tensor("xuint", [NDC, 2 * T_CORE, 128], U16)
        else:
            xu_d = nc.dram_tensor("xu", [NDC, 2 * T_CORE, 128], U16,
                                  kind="ExternalInput")
        w_d = nc.dram_tensor("w", [D, E], F32, kind="ExternalInput")
    elif prec == "f16":
        kind = {} if internal_x else {"kind": "ExternalInput"}
        sfx = "int" if internal_x else ""
        xh_d = nc.dram_tensor("xh" + sfx, xshape, F16, **kind)
        xl_d = nc.dram_tensor("xl" + sfx, xshape, F16, **kind)
        wh_d = nc.dram_tensor("wh", [D, E], F16, kind="ExternalInput")
        wl_d = nc.dram_tensor("wl", [D, E], F16, kind="ExternalInput")
    else:
        if internal_x:
            xt_d = nc.dram_tensor("xtint", xshape, F32)
        else:
            xt_d = nc.dram_tensor("xt", xshape, F32, kind="ExternalInput")
        w_d = nc.dram_tensor("w", [D, E], F32, kind="ExternalInput")
    ids_d = nc.dram_tensor("ids", [T_CORE, TOPK], dt.uint32, kind="ExternalOutput")
    vals_d = nc.dram_tensor("vals", [T_CORE, TOPK], F32, kind="ExternalOutput")

    with tile.TileContext(nc) as tc:
        if mode == "compute":
            tc.race_detector_enabled = False
        with (
            tc.tile_pool(name="xts", bufs=XTS_BUFS) as xts_pool,
            tc.tile_pool(name="wp", bufs=1) as w_pool,
            tc.tile_pool(name="lf", bufs=2) as lf_pool,
            tc.tile_pool(name="sm", bufs=2) as sm_pool,
            tc.tile_pool(name="outp", bufs=2) as out_pool,
            tc.tile_pool(name="gp", bufs=G_BUFS, space="PSUM") as g_psum,
            tc.tile_pool(name="lt", bufs=LT_BUFS, space="PSUM") as lt_psum,
        ):
            ident = w_pool.tile([128, 128], F32, tag="ident")
            make_identity(nc, ident)
            if gemm == "c" and FILL_MM:
                fz = w_pool.tile([128, 64], F32, tag="fz")
                nc.vector.memset(fz[:], 0.0)
            if prec == "f16":
                wh_sb = w_pool.tile([128, NDC, E], F16, tag="wh")
                nc.gpsimd.dma_start(
                    wh_sb[:], wh_d.rearrange("(c p) e -> p c e", p=128))
                wl_sb = w_pool.tile([128, NDC, E], F16, tag="wl")
                nc.gpsimd.dma_start(
                    wl_sb[:], wl_d.rearrange("(c p) e -> p c e", p=128))
            else:
                w_sb = w_pool.tile([128, NDC, E], F32, tag="w")
                nc.gpsimd.dma_start(
                    w_sb[:], w_d.rearrange("(c p) e -> p c e", p=128))

            def alloc_outs():
                i_all = out_pool.tile(
                    [128, T_CORE // 128, TOPK], dt.uint32, tag="i", name="i_all"
                )
                v_all = out_pool.tile(
                    [128, T_CORE // 128, TOPK], F32, tag="v", name="v_all"
                )
                return i_all, v_all

            OUTS = {}

            def softmax_top8(pl, idx):
                """pl: PSUM [128 tok, 64 exp] exact fp32 logits."""
                i_all, v_all = OUTS["i"], OUTS["v"]
                l_sb = sm_pool.tile([128, E], F32, tag="l")
                nc.vector.tensor_copy(l_sb[:], pl[:])
                nmax = sm_pool.tile([128, 1], F32, tag="nm")
                nc.vector.tensor_reduce(
                    nmax[:], l_sb[:], axis=AX.X, op=ALU.max, negate=True,
                )
                e_sb = sm_pool.tile([128, E], F32, tag="e")
                s_sb = sm_pool.tile([128, 1], F32, tag="s")
                nc.scalar.activation(
                    e_sb[:], pl[:], AF.Exp, bias=nmax[:], accum_out=s_sb[:],
                )
                r_sb = sm_pool.tile([128, 1], F32, tag="r")
                nc.vector.reciprocal(r_sb[:], s_sb[:])
                m8 = sm_pool.tile([128, TOPK], F32, tag="m8")
                nc.vector.max(out=m8[:], in_=l_sb[:])
                nc.vector.max_index(
                    out=i_all[:, idx, :], in_max=m8[:], in_values=l_sb[:],
                )
                e8 = sm_pool.tile([128, TOPK], F32, tag="e8")
                nc.scalar.activation(e8[:], m8[:], AF.Exp, bias=nmax[:])
                nc.vector.tensor_scalar(
                    out=v_all[:, idx, :], in0=e8[:], scalar1=r_sb[:],
                    scalar2=None, op0=ALU.mult,
                )

            RINGS = os.environ.get("RINGS", "sc")
            _ENGS = {"s": nc.sync, "c": nc.scalar, "3": nc.gpsimd}

            def load_group(g, tile_, dram, parity):
                for s in range(SPL):
                    eng = _ENGS[RINGS[(g * SPL + s + parity) % len(RINGS)]]
                    eng.dma_start(
                        tile_[:, ds(s * CW, CW), :],
                        dram[g, s, :, :, :],
                    )

            def issue_load(g):
                """Allocate + start this group's loads.  Called one group
                AHEAD of compute so the DMA instructions land on the sync/
                scalar queues BEFORE the previous group's tail (ACT exp)
                instructions — otherwise the scalar-ring DMAs sit FIFO
                behind gemm-dependent activations and DMA serializes with
                compute."""
                if prec == "f16":
                    xtsh = xts_pool.tile([128, NDC, TG], F16, tag="xh")
                    xtsl = xts_pool.tile([128, NDC, TG], F16, tag="xl")
                    if mode != "compute":
                        load_group(g, xtsh, xh_d, g)
                        load_group(g, xtsl, xl_d, g + 1)
                    else:
                        nc.vector.memset(xtsh[:, 0, ds(0, 4)], 0.0)
                        nc.vector.memset(xtsl[:, 0, ds(0, 4)], 0.0)
                    return (xtsh, xtsl)
                xts = xts_pool.tile([128, NDC, TG], F32, tag="xts")
                if mode == "compute":
                    nc.vector.memset(xts[:, 0, ds(0, 4)], 0.0)
                elif layout == "xbar":
                    for c in range(NDC):
                        # all xbar transposes on ONE HWDGE ring: concurrent
                        # transposes from sync+scalar rings corrupt data
                        nc.sync.dma_start(
                            xts[:, c, :].bitcast(dt.uint16),
                            xu_d[c, ds(2 * g * TG, 2 * TG), :],
                            transpose=True,
                        )
                else:
                    load_group(g, xts, xt_d, g)
                return xts

            def body():
                OUTS["i"], OUTS["v"] = alloc_outs()
                pend = issue_load(0)
                for g in range(N_GROUPS):
                    cur = pend
                    if g + 1 < N_GROUPS:
                        pend = issue_load(g + 1)
                    if prec == "f16":
                        xtsh, xtsl = cur
                        if mode == "dma":
                            continue
                        pg = g_psum.tile([64, TG], F32, tag="g")
                        n_mm = NDC * 3
                        i_mm = 0
                        for dc in range(NDC):
                            for (wt, xt_t) in ((wh_sb, xtsh), (wh_sb, xtsl),
                                               (wl_sb, xtsh)):
                                nc.tensor.matmul(
                                    pg[:], wt[:, dc, :], xt_t[:, dc, :],
                                    start=(i_mm == 0), stop=(i_mm == n_mm - 1),
                                )
                                i_mm += 1
                        lf_sb = lf_pool.tile([64, TG], F32, tag="lf")
                        nc.vector.tensor_scalar(
                            out=lf_sb[:], in0=pg[:], scalar1=RESCALE,
                            scalar2=None, op0=ALU.mult,
                        )
                        for tt in range(TPG):
                            pl = lt_psum.tile([128, E], F32, tag="lt")
                            nc.tensor.matmul(
                                pl[:], lf_sb[:, ds(tt * 128, 128)],
                                ident[:64, :64], is_transpose=True,
                            )
                            softmax_top8(pl, g * TPG + tt)
                        continue
                    xts = cur
                    if mode == "dma":
                        continue
                    if gemm == "x":
                        for tt in range(TPG):
                            pa = g_psum.tile([128, E], F32, tag=f"pa{tt % 2}")
                            for dc in range(NDC):
                                nc.tensor.matmul(
                                    pa[:], xts[:, dc, ds(tt * 128, 128)],
                                    w_sb[:, dc, :],
                                    start=(dc == 0), stop=(dc == NDC - 1),
                                )
                            softmax_top8(pa, g * TPG + tt)
                    elif gemm == "c":
                        # 128x64 column tiling: two independent PE tiles
                        # (cols 0-63 / 64-127) stream W concurrently, one
                        # 64-token stationary half each.  Same per-token
                        # contraction order as gemm="x" -> bit-exact.
                        for tt in range(TPG):
                            pa = g_psum.tile([128, E], F32, tag=f"pa{tt % 2}")
                            for dc in range(NDC):
                                for h in range(2):
                                    nc.tensor.matmul(
                                        pa[h * 64:(h + 1) * 64, :],
                                        xts[:, dc, ds(tt * 128 + h * 64, 64)],
                                        w_sb[:, dc, :],
                                        start=(dc == 0),
                                        stop=(dc == NDC - 1),
                                        tile_position=(0, h * 64),
                                    )
                            softmax_top8(pa, g * TPG + tt)
                        if FILL_MM:
                            fpa = lt_psum.tile([128, 64], F32, tag="fpa")
                            for _ in range(FILL_MM):
                                nc.tensor.matmul(
                                    fpa[:64, :], fz[:, :], fz[:, :],
                                    start=True, stop=True,
                                    tile_position=(0, 0),
                                )
                    else:
                        pg = g_psum.tile([64, TG], F32, tag="g")
                        for dc in range(NDC):
                            nc.tensor.matmul(
                                pg[:], w_sb[:, dc, :], xts[:, dc, :],
                                start=(dc == 0), stop=(dc == NDC - 1),
                            )
                        lf_sb = lf_pool.tile([64, TG], F32, tag="lf")
                        nc.vector.tensor_copy(lf_sb[:], pg[:])
                        for tt in range(TPG):
                            pl = lt_psum.tile([128, E], F32, tag="lt")
                            nc.tensor.matmul(
                                pl[:], lf_sb[:, ds(tt * 128, 128)],
                                ident[:64, :64], is_transpose=True,
                            )
                            softmax_top8(pl, g * TPG + tt)
                i_all, v_all = OUTS["i"], OUTS["v"]
                if mode == "dma":
                    nc.vector.memset(i_all[:], 0)
                    nc.vector.memset(v_all[:], 0.0)
                # outputs ride the SWDGE (gpsimd) ring: HWDGE rings execute
                # FIFO per ring, so putting these on sync would stall the
                # next rep's x sub-DMAs behind the HBM write round trip
                nc.gpsimd.dma_start(
                    ids_d.rearrange("(q p) k -> p q k", p=128), i_all[:]
                )
                nc.gpsimd.dma_start(
                    vals_d.rearrange("(q p) k -> p q k", p=128), v_all[:]
                )

            if reps == 1:
                body()
            else:
                with tc.For_i(0, reps, 1):
                    body()

    nc.finalize()
    return nc


# Default "pt32c": host-transposed plain-DMA layout + 128x64 PE column
# tiling (two tiles stream W concurrently, one 64-token stationary half
# each).  Bit-exact vs the reference (0 id mismatches, 0.0 rel err): each
# token's dot product keeps the exact gemm="x" contraction order.
# ~121 us/rep under heavy co-tenant load vs 250+ for the old tx32 default;
# ~100-120 us on a quiet device (DMA floor 32MiB @ ~360-380 GB/s ~ 89-94 us).
# Alternatives (env MOE_VARIANT):
#   tx32  — device DMA-transpose layout, bit-exact, but single-ring
#           transpose DMA caps at ~261 GB/s and PE stalls throttle HAM
#   pt32  — plain-DMA layout, bit-exact full-array gemm; PE-bound at
#           1024 fp32 HI/LO pair instrs/rep (~109 us warm, ~218 throttled)
#   pt32w — W-stationary fp32 gemm, NOT bit-identical to the reference
#           (flips the order of two half-ulp-tied experts on 1/16384
#           tokens; vals rel err ~2e-6)
#   pt16  — fp16 hi/lo 3-term gemm, same single tie-token caveat
VARIANT = os.environ.get("MOE_VARIANT", "pt32c")


def _get_nc(reps: int = 1, internal_x: bool = False, mode: str = "full",
            variant: str | None = None):
    variant = variant or VARIANT
    key = (reps, internal_x, mode, variant)
    if key not in _cache:
        gemm = "w" if variant.endswith("w") else ("c" if variant.endswith("c") else "x")
        prec = "f16" if variant == "pt16" else "f32"
        layout = "xbar" if variant.startswith("tx") else "plain"
        _cache[key] = build_tx(reps, internal_x, mode, gemm=gemm, prec=prec,
                               layout=layout)
    return _cache[key]


def _transpose_xt(x5: np.ndarray) -> np.ndarray:
    """[N_CORES, N_GROUPS, TG, NDC, 128] -> [N_CORES, N_GROUPS, SPL, 128, CW, TG]."""
    spl = int(os.environ.get("DMA_SPLIT", "4"))
    x6 = x5.reshape(N_CORES, N_GROUPS, TG, spl, NDC // spl, 128)
    return np.ascontiguousarray(x6.transpose(0, 1, 3, 5, 4, 2))


def bench_in_maps(w: np.ndarray):
    """in_maps for the internal-x timed variant (x DRAM tensors internal)."""
    w = np.ascontiguousarray(np.asarray(w), dtype=np.float32)
    if VARIANT == "pt16":
        ws = w * (2.0 ** SWL)
        wh = ws.astype(np.float16)
        wl = (ws - wh.astype(np.float32)).astype(np.float16)
        return [{"wh": wh, "wl": wl} for _ in range(N_CORES)]
    return [{"w": w} for _ in range(N_CORES)]


def _to_xu(x: np.ndarray) -> np.ndarray:
    """[N_TOKENS, D] f32 -> [N_CORES, NDC, 2*T_CORE, 128] u16 interleaved."""
    xv = x.view(np.uint16).reshape(N_CORES, T_CORE, NDC, 128, 2)
    return np.ascontiguousarray(
        xv.transpose(0, 2, 1, 4, 3).reshape(N_CORES, NDC, 2 * T_CORE, 128)
    )


def kernel(x: np.ndarray, W_g: np.ndarray):
    from concourse.bass_utils import run_bass_kernel_spmd

    x = np.ascontiguousarray(np.asarray(x), dtype=np.float32)
    w = np.ascontiguousarray(np.asarray(W_g), dtype=np.float32)
    nc = _get_nc(1)
    if VARIANT.startswith("tx"):
        xu = _to_xu(x)
        in_maps = [{"xu": xu[c], "w": w} for c in range(N_CORES)]
        res = run_bass_kernel_spmd(nc, in_maps, core_ids=list(range(N_CORES)))
        ids = np.concatenate([res.results[c]["ids"] for c in range(N_CORES)], axis=0)
        vals = np.concatenate([res.results[c]["vals"] for c in range(N_CORES)], axis=0)
        return ids.astype(np.int32), vals
    x5 = x.reshape(N_CORES, N_GROUPS, TG, NDC, 128)
    if VARIANT == "pt16":
        xs = x5 * (2.0 ** SXL)
        xh = xs.astype(np.float16)
        xl = (xs - xh.astype(np.float32)).astype(np.float16)
        xht = _transpose_xt(xh)
        xlt = _transpose_xt(xl)
        ws = w * (2.0 ** SWL)
        wh = ws.astype(np.float16)
        wl = (ws - wh.astype(np.float32)).astype(np.float16)
        in_maps = [
            {"xh": xht[c], "xl": xlt[c], "wh": wh, "wl": wl}
            for c in range(N_CORES)
        ]
    else:
        xt = _transpose_xt(x5)
        in_maps = [{"xt": xt[c], "w": w} for c in range(N_CORES)]
    res = run_bass_kernel_spmd(nc, in_maps, core_ids=list(range(N_CORES)))
    ids = np.concatenate([res.results[c]["ids"] for c in range(N_CORES)], axis=0)
    vals = np.concatenate([res.results[c]["vals"] for c in range(N_CORES)], axis=0)
    return ids.astype(np.int32), vals



# revision 20
# speedup vs baseline: 1.0559x; 1.0413x over previous
"""MoE gate kernel for Trainium2 (8 NeuronCores).

reference math: logits = x @ W_g; probs = softmax(logits); top-8 (vals, ids).

Strategy (token-parallel, 2048 tokens/core, default variant "pt32c"):
  - x is reformatted host-side into a transposed layout
    XT[g, p, c, t] = x[g*TG + t, c*128 + p]  so the device streams x^T
    tiles [128 d, NDC, TG=256] with plain contiguous DMA (1 MiB sub-DMAs
    on alternating sync/scalar HWDGE rings, 4-deep group prefetch, no
    on-chip transposes at all).
  - fp32 PE gemm (exact), accumulated over 32 k-chunks in PSUM, using
    128x64 column tiling: the PE array is split into two independent
    64-col tiles (tile_position (0,0)/(0,64)), each holding a 64-token
    stationary x-chunk; both stream the same moving W [128,64]
    concurrently, doubling throughput of the fp32 HI/LO pair stream.
    Per token the contraction order is identical to the full-array
    gemm="x" -> bit-exact vs the jax reference (0 id mismatches).
  - top-8 selection on exact fp32 logits via DVE max8/max_index
  - vals = exp(top8_logit - max) * 1/sum(exp(logits - max))
"""
import os
import sys
sys.path.insert(0, "/opt/trn_rl_repo")
import numpy as np

N_TOKENS = 16384
D = 4096
E = 64
TOPK = 8
N_CORES = 8
T_CORE = N_TOKENS // N_CORES   # 2048
TG = int(os.environ.get("MOE_TG", "256"))  # tokens per group
N_GROUPS = T_CORE // TG        # 4
TPG = TG // 128                # token-tiles per group
NDC = D // 128                 # 32 k-chunks

_cache = {}


SXL = 4          # log2 scale for x in the f16 split
SWL = 10         # log2 scale for W_g in the f16 split
RESCALE = 2.0 ** (-(SXL + SWL))


def build_tx(reps: int = 1, internal_x: bool = False, mode: str = "full",
             gemm: str = "x", prec: str = "f32", layout: str = "plain"):
    import concourse.mybir as mybir
    import concourse.tile as tile
    from concourse import bacc
    from concourse.bass import ds
    from concourse.masks import make_identity

    dt = mybir.dt
    F32 = dt.float32
    F16 = dt.float16
    AF = mybir.ActivationFunctionType
    AX = mybir.AxisListType
    ALU = mybir.AluOpType

    XTS_BUFS = int(os.environ.get("XTS_BUFS", "4"))
    G_BUFS = int(os.environ.get("G_BUFS", "2"))
    LT_BUFS = int(os.environ.get("LT_BUFS", "2"))
    DMA_SPLIT = int(os.environ.get("DMA_SPLIT", "4"))  # DMAs per group
    # dummy 64-col matmuls appended per group: keep the PE HAM-warm across
    # DMA-paced group boundaries instead of micro-idling (throttle to 1.2GHz)
    FILL_MM = int(os.environ.get("FILL_MM", "0"))

    nc = bacc.Bacc("TRN2", target_bir_lowering=False, debug=False)
    SPL = DMA_SPLIT
    CW = NDC // SPL
    xshape = [N_GROUPS, SPL, 128, CW, TG]
    if layout == "xbar":
        U16 = dt.uint16
        if internal_x:
            xu_d = nc.dram_tensor("xuint", [NDC, 2 * T_CORE, 128], U16)
        else:
            xu_d = nc.dram_tensor("xu", [NDC, 2 * T_CORE, 128], U16,
                                  kind="ExternalInput")
        w_d = nc.dram_tensor("w", [D, E], F32, kind="ExternalInput")
    elif prec == "f16":
        kind = {} if internal_x else {"kind": "ExternalInput"}
        sfx = "int" if internal_x else ""
        xh_d = nc.dram_tensor("xh" + sfx, xshape, F16, **kind)
        xl_d = nc.dram_tensor("xl" + sfx, xshape, F16, **kind)
        wh_d = nc.dram_tensor("wh", [D, E], F16, kind="ExternalInput")
        wl_d = nc.dram_tensor("wl", [D, E], F16, kind="ExternalInput")
    else:
        if internal_x:
            xt_d = nc.dram_tensor("xtint", xshape, F32)
        else:
            xt_d = nc.dram_tensor("xt", xshape, F32, kind="ExternalInput")
        w_d = nc.dram_tensor("w", [D, E], F32, kind="ExternalInput")
    ids_d = nc.dram_tensor("ids", [T_CORE, TOPK], dt.uint32, kind="ExternalOutput")
    vals_d = nc.dram_tensor("vals", [T_CORE, TOPK], F32, kind="ExternalOutput")

    with tile.TileContext(nc) as tc:
        if mode == "compute":
            tc.race_detector_enabled = False
        with (
            tc.tile_pool(name="xts", bufs=XTS_BUFS) as xts_pool,
            tc.tile_pool(name="wp", bufs=1) as w_pool,
            tc.tile_pool(name="lf", bufs=2) as lf_pool,
            tc.tile_pool(name="sm", bufs=2) as sm_pool,
            tc.tile_pool(name="outp", bufs=2) as out_pool,
            tc.tile_pool(name="gp", bufs=G_BUFS, space="PSUM") as g_psum,
            tc.tile_pool(name="lt", bufs=LT_BUFS, space="PSUM") as lt_psum,
        ):
            ident = w_pool.tile([128, 128], F32, tag="ident")
            make_identity(nc, ident)
            if gemm == "c" and FILL_MM:
                fz = w_pool.tile([128, 64], F32, tag="fz")
                nc.vector.memset(fz[:], 0.0)
            if prec == "f16":
                wh_sb = w_pool.tile([128, NDC, E], F16, tag="wh")
                nc.gpsimd.dma_start(
                    wh_sb[:], wh_d.rearrange("(c p) e -> p c e", p=128))
                wl_sb = w_pool.tile([128, NDC, E], F16, tag="wl")
                nc.gpsimd.dma_start(
                    wl_sb[:], wl_d.rearrange("(c p) e -> p c e", p=128))
            else:
                w_sb = w_pool.tile([128, NDC, E], F32, tag="w")
                nc.gpsimd.dma_start(
                    w_sb[:], w_d.rearrange("(c p) e -> p c e", p=128))

            def alloc_outs():
                i_all = out_pool.tile(
                    [128, T_CORE // 128, TOPK], dt.uint32, tag="i", name="i_all"
                )
                v_all = out_pool.tile(
                    [128, T_CORE // 128, TOPK], F32, tag="v", name="v_all"
                )
                return i_all, v_all

            OUTS = {}

            def softmax_top8(pl, idx):
                """pl: PSUM [128 tok, 64 exp] exact fp32 logits."""
                i_all, v_all = OUTS["i"], OUTS["v"]
                l_sb = sm_pool.tile([128, E], F32, tag="l")
                nc.vector.tensor_copy(l_sb[:], pl[:])
                nmax = sm_pool.tile([128, 1], F32, tag="nm")
                nc.vector.tensor_reduce(
                    nmax[:], l_sb[:], axis=AX.X, op=ALU.max, negate=True,
                )
                e_sb = sm_pool.tile([128, E], F32, tag="e")
                s_sb = sm_pool.tile([128, 1], F32, tag="s")
                nc.scalar.activation(
                    e_sb[:], pl[:], AF.Exp, bias=nmax[:], accum_out=s_sb[:],
                )
                r_sb = sm_pool.tile([128, 1], F32, tag="r")
                nc.vector.reciprocal(r_sb[:], s_sb[:])
                m8 = sm_pool.tile([128, TOPK], F32, tag="m8")
                nc.vector.max(out=m8[:], in_=l_sb[:])
                nc.vector.max_index(
                    out=i_all[:, idx, :], in_max=m8[:], in_values=l_sb[:],
                )
                e8 = sm_pool.tile([128, TOPK], F32, tag="e8")
                nc.scalar.activation(e8[:], m8[:], AF.Exp, bias=nmax[:])
                nc.vector.tensor_scalar(
                    out=v_all[:, idx, :], in0=e8[:], scalar1=r_sb[:],
                    scalar2=None, op0=ALU.mult,
                )

            RINGS = os.environ.get("RINGS", "sc")
            _ENGS = {"s": nc.sync, "c": nc.scalar, "3": nc.gpsimd}

            def load_group(g, tile_, dram, parity):
                for s in range(SPL):
                    eng = _ENGS[RINGS[(g * SPL + s + parity) % len(RINGS)]]
                    eng.dma_start(
                        tile_[:, ds(s * CW, CW), :],
                        dram[g, s, :, :, :],
                    )

            def issue_load(g):
                """Allocate + start this group's loads.  Called one group
                AHEAD of compute so the DMA instructions land on the sync/
                scalar queues BEFORE the previous group's tail (ACT exp)
                instructions — otherwise the scalar-ring DMAs sit FIFO
                behind gemm-dependent activations and DMA serializes with
                compute."""
                if prec == "f16":
                    xtsh = xts_pool.tile([128, NDC, TG], F16, tag="xh")
                    xtsl = xts_pool.tile([128, NDC, TG], F16, tag="xl")
                    if mode != "compute":
                        load_group(g, xtsh, xh_d, g)
                        load_group(g, xtsl, xl_d, g + 1)
                    else:
                        nc.vector.memset(xtsh[:, 0, ds(0, 4)], 0.0)
                        nc.vector.memset(xtsl[:, 0, ds(0, 4)], 0.0)
                    return (xtsh, xtsl)
                xts = xts_pool.tile([128, NDC, TG], F32, tag="xts")
                if mode == "compute":
                    nc.vector.memset(xts[:, 0, ds(0, 4)], 0.0)
                elif layout == "xbar":
                    for c in range(NDC):
                        # all xbar transposes on ONE HWDGE ring: concurrent
                        # transposes from sync+scalar rings corrupt data
                        nc.sync.dma_start(
                            xts[:, c, :].bitcast(dt.uint16),
                            xu_d[c, ds(2 * g * TG, 2 * TG), :],
                            transpose=True,
                        )
                else:
                    load_group(g, xts, xt_d, g)
                return xts

            def body():
                OUTS["i"], OUTS["v"] = alloc_outs()
                pend = issue_load(0)
                for g in range(N_GROUPS):
                    cur = pend
                    if g + 1 < N_GROUPS:
                        pend = issue_load(g + 1)
                    if prec == "f16":
                        xtsh, xtsl = cur
                        if mode == "dma":
                            continue
                        pg = g_psum.tile([64, TG], F32, tag="g")
                        n_mm = NDC * 3
                        i_mm = 0
                        for dc in range(NDC):
                            for (wt, xt_t) in ((wh_sb, xtsh), (wh_sb, xtsl),
                                               (wl_sb, xtsh)):
                                nc.tensor.matmul(
                                    pg[:], wt[:, dc, :], xt_t[:, dc, :],
                                    start=(i_mm == 0), stop=(i_mm == n_mm - 1),
                                )
                                i_mm += 1
                        lf_sb = lf_pool.tile([64, TG], F32, tag="lf")
                        nc.vector.tensor_scalar(
                            out=lf_sb[:], in0=pg[:], scalar1=RESCALE,
                            scalar2=None, op0=ALU.mult,
                        )
                        for tt in range(TPG):
                            pl = lt_psum.tile([128, E], F32, tag="lt")
                            nc.tensor.matmul(
                                pl[:], lf_sb[:, ds(tt * 128, 128)],
                                ident[:64, :64], is_transpose=True,
                            )
                            softmax_top8(pl, g * TPG + tt)
                        continue
                    xts = cur
                    if mode == "dma":
                        continue
                    if gemm == "x":
                        for tt in range(TPG):
                            pa = g_psum.tile([128, E], F32, tag=f"pa{tt % 2}")
                            for dc in range(NDC):
                                nc.tensor.matmul(
                                    pa[:], xts[:, dc, ds(tt * 128, 128)],
                                    w_sb[:, dc, :],
                                    start=(dc == 0), stop=(dc == NDC - 1),
                                )
                            softmax_top8(pa, g * TPG + tt)
                    elif gemm == "c":
                        # 128x64 column tiling: two independent PE tiles
                        # (cols 0-63 / 64-127) stream W concurrently, one
                        # 64-token stationary half each.  Same per-token
                        # contraction order as gemm="x" -> bit-exact.
                        for tt in range(TPG):
                            pa = g_psum.tile([128, E], F32, tag=f"pa{tt % 2}")
                            for dc in range(NDC):
                                for h in range(2):
                                    nc.tensor.matmul(
                                        pa[h * 64:(h + 1) * 64, :],
                                        xts[:, dc, ds(tt * 128 + h * 64, 64)],
                                        w_sb[:, dc, :],
                                        start=(dc == 0),
                                        stop=(dc == NDC - 1),
                                        tile_position=(0, h * 64),
                                    )
                            softmax_top8(pa, g * TPG + tt)
                        if FILL_MM:
                            fpa = lt_psum.tile([128, 64], F32, tag="fpa")
                            for _ in range(FILL_MM):
                                nc.tensor.matmul(
                                    fpa[:64, :], fz[:, :], fz[:, :],
                                    start=True, stop=True,
                                    tile_position=(0, 0),
                                )
                    else:
                        pg = g_psum.tile([64, TG], F32, tag="g")
                        for dc in range(NDC):
                            nc.tensor.matmul(
                                pg[:], w_sb[:, dc, :], xts[:, dc, :],
                                start=(dc == 0), stop=(dc == NDC - 1),
                            )
                        lf_sb = lf_pool.tile([64, TG], F32, tag="lf")
                        nc.vector.tensor_copy(lf_sb[:], pg[:])
                        for tt in range(TPG):
                            pl = lt_psum.tile([128, E], F32, tag="lt")
                            nc.tensor.matmul(
                                pl[:], lf_sb[:, ds(tt * 128, 128)],
                                ident[:64, :64], is_transpose=True,
                            )
                            softmax_top8(pl, g * TPG + tt)
                i_all, v_all = OUTS["i"], OUTS["v"]
                if mode == "dma":
                    nc.vector.memset(i_all[:], 0)
                    nc.vector.memset(v_all[:], 0.0)
                # outputs ride the SWDGE (gpsimd) ring: HWDGE rings execute
                # FIFO per ring, so putting these on sync would stall the
                # next rep's x sub-DMAs behind the HBM write round trip
                nc.gpsimd.dma_start(
                    ids_d.rearrange("(q p) k -> p q k", p=128), i_all[:]
                )
                nc.gpsimd.dma_start(
                    vals_d.rearrange("(q p) k -> p q k", p=128), v_all[:]
                )

            if reps == 1:
                body()
            else:
                # unroll the hardware rep loop: the For_i boundary costs a
                # cross-engine rendezvous (~5-10us of Tensor COMPARE_BRANCH
                # + GpSimd DRAIN waits per iteration) — amortize it 4x
                U = int(os.environ.get("UNROLL", "4"))
                while reps % U:
                    U -= 1
                with tc.For_i(0, reps // U, 1):
                    for _ in range(U):
                        body()

    nc.finalize()
    return nc


# Default "pt32c": host-transposed plain-DMA layout + 128x64 PE column
# tiling (two tiles stream W concurrently, one 64-token stationary half
# each).  Bit-exact vs the reference (0 id mismatches, 0.0 rel err): each
# token's dot product keeps the exact gemm="x" contraction order.
# ~121 us/rep under heavy co-tenant load vs 250+ for the old tx32 default;
# ~100-120 us on a quiet device (DMA floor 32MiB @ ~360-380 GB/s ~ 89-94 us).
# Alternatives (env MOE_VARIANT):
#   tx32  — device DMA-transpose layout, bit-exact, but single-ring
#           transpose DMA caps at ~261 GB/s and PE stalls throttle HAM
#   pt32  — plain-DMA layout, bit-exact full-array gemm; PE-bound at
#           1024 fp32 HI/LO pair instrs/rep (~109 us warm, ~218 throttled)
#   pt32w — W-stationary fp32 gemm, NOT bit-identical to the reference
#           (flips the order of two half-ulp-tied experts on 1/16384
#           tokens; vals rel err ~2e-6)
#   pt16  — fp16 hi/lo 3-term gemm, same single tie-token caveat
VARIANT = os.environ.get("MOE_VARIANT", "pt32c")


def _get_nc(reps: int = 1, internal_x: bool = False, mode: str = "full",
            variant: str | None = None):
    variant = variant or VARIANT
    key = (reps, internal_x, mode, variant)
    if key not in _cache:
        gemm = "w" if variant.endswith("w") else ("c" if variant.endswith("c") else "x")
        prec = "f16" if variant == "pt16" else "f32"
        layout = "xbar" if variant.startswith("tx") else "plain"
        _cache[key] = build_tx(reps, internal_x, mode, gemm=gemm, prec=prec,
                               layout=layout)
    return _cache[key]


def _transpose_xt(x5: np.ndarray) -> np.ndarray:
    """[N_CORES, N_GROUPS, TG, NDC, 128] -> [N_CORES, N_GROUPS, SPL, 128, CW, TG]."""
    spl = int(os.environ.get("DMA_SPLIT", "4"))
    x6 = x5.reshape(N_CORES, N_GROUPS, TG, spl, NDC // spl, 128)
    return np.ascontiguousarray(x6.transpose(0, 1, 3, 5, 4, 2))


def bench_in_maps(w: np.ndarray):
    """in_maps for the internal-x timed variant (x DRAM tensors internal)."""
    w = np.ascontiguousarray(np.asarray(w), dtype=np.float32)
    if VARIANT == "pt16":
        ws = w * (2.0 ** SWL)
        wh = ws.astype(np.float16)
        wl = (ws - wh.astype(np.float32)).astype(np.float16)
        return [{"wh": wh, "wl": wl} for _ in range(N_CORES)]
    return [{"w": w} for _ in range(N_CORES)]


def _to_xu(x: np.ndarray) -> np.ndarray:
    """[N_TOKENS, D] f32 -> [N_CORES, NDC, 2*T_CORE, 128] u16 interleaved."""
    xv = x.view(np.uint16).reshape(N_CORES, T_CORE, NDC, 128, 2)
    return np.ascontiguousarray(
        xv.transpose(0, 2, 1, 4, 3).reshape(N_CORES, NDC, 2 * T_CORE, 128)
    )


def kernel(x: np.ndarray, W_g: np.ndarray):
    from concourse.bass_utils import run_bass_kernel_spmd

    x = np.ascontiguousarray(np.asarray(x), dtype=np.float32)
    w = np.ascontiguousarray(np.asarray(W_g), dtype=np.float32)
    nc = _get_nc(1)
    if VARIANT.startswith("tx"):
        xu = _to_xu(x)
        in_maps = [{"xu": xu[c], "w": w} for c in range(N_CORES)]
        res = run_bass_kernel_spmd(nc, in_maps, core_ids=list(range(N_CORES)))
        ids = np.concatenate([res.results[c]["ids"] for c in range(N_CORES)], axis=0)
        vals = np.concatenate([res.results[c]["vals"] for c in range(N_CORES)], axis=0)
        return ids.astype(np.int32), vals
    x5 = x.reshape(N_CORES, N_GROUPS, TG, NDC, 128)
    if VARIANT == "pt16":
        xs = x5 * (2.0 ** SXL)
        xh = xs.astype(np.float16)
        xl = (xs - xh.astype(np.float32)).astype(np.float16)
        xht = _transpose_xt(xh)
        xlt = _transpose_xt(xl)
        ws = w * (2.0 ** SWL)
        wh = ws.astype(np.float16)
        wl = (ws - wh.astype(np.float32)).astype(np.float16)
        in_maps = [
            {"xh": xht[c], "xl": xlt[c], "wh": wh, "wl": wl}
            for c in range(N_CORES)
        ]
    else:
        xt = _transpose_xt(x5)
        in_maps = [{"xt": xt[c], "w": w} for c in range(N_CORES)]
    res = run_bass_kernel_spmd(nc, in_maps, core_ids=list(range(N_CORES)))
    ids = np.concatenate([res.results[c]["ids"] for c in range(N_CORES)], axis=0)
    vals = np.concatenate([res.results[c]["vals"] for c in range(N_CORES)], axis=0)
    return ids.astype(np.int32), vals

